# revision 2
# baseline (speedup 1.0000x reference)
"""Trainium2 Bass kernel for nn_Net_25469156065564.

Pipeline (per core, 64 samples, data-parallel over batch):
  1. word embedding rows gathered via indirect DMA into a token-major
     staging buffer (bf16), then PE-transposed into a channel-major
     padded-column layout (sample stride 112, lead 16).
  2. weighted plane = wordT * sdp (sdp host-replicated across partitions).
  3. pos planes built as one-hot matmuls: posT = pos_emb.T @ onehot,
     with the one-hot encoding of the (0..203) indices built on host.
  4. conv1 (3 parallel convs, kernel (3,k) k=1,3,5) as 54 accumulating
     matmuls per 448-column tile; relu+maxpool2 fused into the PSUM
     evacuation.
  5. conv2a (1x1) + conv2b (1x3) + maxpool50 the same way on a
     stride-52 padded layout.
  6. 3-layer MLP in fp32; output [15, 64] per core, assembled on host.
"""
import os
import sys

for _p in ("/opt/trn_rl_repo", "/root/.axon_site/_ro/trn_rl_repo"):
    if os.path.isdir(_p) and _p not in sys.path:
        sys.path.append(_p)

import numpy as np
import ml_dtypes

import concourse.bass as bass
import concourse.bacc as bacc
import concourse.mybir as mybir
import concourse.tile as tile
from concourse.bass_utils import run_bass_kernel_spmd
from concourse.masks import make_identity

BF16 = ml_dtypes.bfloat16
NCORES = 8
B, L, V, WD, PD, PLEN = 512, 100, 400000, 250, 125, 204
BC = B // NCORES          # samples per core
SS = 112                  # conv1 column stride per sample
LEAD = 16                 # leading zero columns (32B aligned)
NCOL1 = LEAD + BC * SS + SS          # 7296
G1 = 4                    # samples per conv1 column tile
NT1 = BC // G1            # 16
W1 = G1 * SS              # 448
SS2 = 52                  # conv2 column stride per sample
NCOL2 = 3332
G2 = 8
NT2 = BC // G2            # 8
W2 = G2 * SS2             # 416
STG_SLOTS = 32            # staging ring depth (samples)

_cache = {}


def _conv1_tap_plan():
    """(weight_index, plane, chunk, shift) per tap, grouped by conv."""
    plan = []
    wi = 0
    for k, pw in ((1, 0), (3, 1), (5, 2)):
        taps = []
        for dw in range(k):
            d = dw - pw
            for plane in range(3):        # 0=word, 1=pos, 2=weighted
                for c in range(2):
                    taps.append((wi, plane, c, d))
                    wi += 1
        plan.append(taps)
    return plan


def _build_nc():
    nc = bacc.Bacc(trn_type="TRN2", target_bir_lowering=False, debug=False,
                   num_devices=NCORES)
    dt = mybir.dt
    word_t = nc.dram_tensor("word_t", [V, 256], dt.bfloat16, kind="ExternalInput").ap()
    tok_idx = nc.dram_tensor("tok_idx", [SS, BC], dt.int32, kind="ExternalInput").ap()
    sdpb = nc.dram_tensor("sdpb", [128, NCOL1], dt.bfloat16, kind="ExternalInput").ap()
    oh1 = nc.dram_tensor("oh1", [204, NCOL1], dt.bfloat16, kind="ExternalInput").ap()
    oh2 = nc.dram_tensor("oh2", [204, NCOL1], dt.bfloat16, kind="ExternalInput").ap()
    w1_d = nc.dram_tensor("w1", [128, 54 * 128], dt.bfloat16, kind="ExternalInput").ap()
    p1e_d = nc.dram_tensor("p1e", [102, 2 * 128], dt.bfloat16, kind="ExternalInput").ap()
    p2e_d = nc.dram_tensor("p2e", [102, 2 * 128], dt.bfloat16, kind="ExternalInput").ap()
    w2a_d = nc.dram_tensor("w2a", [100, 6 * 128], dt.bfloat16, kind="ExternalInput").ap()
    w2b_d = nc.dram_tensor("w2b", [100, 18 * 128], dt.bfloat16, kind="ExternalInput").ap()
    d1w_d = nc.dram_tensor("d1w", [100, 3 * 100], dt.float32, kind="ExternalInput").ap()
    d2w_d = nc.dram_tensor("d2w", [100, 50], dt.float32, kind="ExternalInput").ap()
    d3w_d = nc.dram_tensor("d3w", [50, 15], dt.float32, kind="ExternalInput").ap()
    b1_d = nc.dram_tensor("b1", [128, 3], dt.float32, kind="ExternalInput").ap()
    b2a_d = nc.dram_tensor("b2a", [100, 2], dt.float32, kind="ExternalInput").ap()
    b2b_d = nc.dram_tensor("b2b", [100, 3], dt.float32, kind="ExternalInput").ap()
    d1b_d = nc.dram_tensor("d1b", [100, 1], dt.float32, kind="ExternalInput").ap()
    d2b_d = nc.dram_tensor("d2b", [50, 1], dt.float32, kind="ExternalInput").ap()
    d3b_d = nc.dram_tensor("d3b", [15, 1], dt.float32, kind="ExternalInput").ap()
    out_d = nc.dram_tensor("out", [15, BC], dt.float32, kind="ExternalOutput").ap()

    plan = _conv1_tap_plan()

    with tile.TileContext(nc) as tc:
        with (
            tc.tile_pool(name="const", bufs=1) as cpool,
            tc.tile_pool(name="planes", bufs=1) as ppool,
            tc.tile_pool(name="work", bufs=1) as wpool,
            tc.tile_pool(name="stream", bufs=3) as spool,
        ):
            ident = cpool.tile([128, 128], dt.bfloat16)
            make_identity(nc, ident)
            w1_sb = cpool.tile([128, 54 * 128], dt.bfloat16)
            nc.sync.dma_start(out=w1_sb, in_=w1_d)
            p1e_sb = cpool.tile([102, 2 * 128], dt.bfloat16)
            nc.sync.dma_start(out=p1e_sb, in_=p1e_d)
            p2e_sb = cpool.tile([102, 2 * 128], dt.bfloat16)
            nc.sync.dma_start(out=p2e_sb, in_=p2e_d)
            w2a_sb = cpool.tile([100, 6 * 128], dt.bfloat16)
            nc.sync.dma_start(out=w2a_sb, in_=w2a_d)
            w2b_sb = cpool.tile([100, 18 * 128], dt.bfloat16)
            nc.sync.dma_start(out=w2b_sb, in_=w2b_d)
            d1w_sb = cpool.tile([100, 3 * 100], dt.float32)
            nc.sync.dma_start(out=d1w_sb, in_=d1w_d)
            d2w_sb = cpool.tile([100, 50], dt.float32)
            nc.sync.dma_start(out=d2w_sb, in_=d2w_d)
            d3w_sb = cpool.tile([50, 15], dt.float32)
            nc.sync.dma_start(out=d3w_sb, in_=d3w_d)
            b1_sb = cpool.tile([128, 3], dt.float32)
            nc.sync.dma_start(out=b1_sb, in_=b1_d)
            b2a_sb = cpool.tile([100, 2], dt.float32)
            nc.sync.dma_start(out=b2a_sb, in_=b2a_d)
            b2b_sb = cpool.tile([100, 3], dt.float32)
            nc.sync.dma_start(out=b2b_sb, in_=b2b_d)
            d1b_sb = cpool.tile([100, 1], dt.float32)
            nc.sync.dma_start(out=d1b_sb, in_=d1b_d)
            d2b_sb = cpool.tile([50, 1], dt.float32)
            nc.sync.dma_start(out=d2b_sb, in_=d2b_d)
            d3b_sb = cpool.tile([15, 1], dt.float32)
            nc.sync.dma_start(out=d3b_sb, in_=d3b_d)
            idx_sb = cpool.tile([SS, BC], dt.int32)
            nc.sync.dma_start(out=idx_sb, in_=tok_idx)
            sdpb_sb = cpool.tile([128, NCOL1], dt.bfloat16)
            nc.sync.dma_start(out=sdpb_sb, in_=sdpb)

            wordT = ppool.tile([128, 2, NCOL1], dt.bfloat16)
            wghT = ppool.tile([128, 2, NCOL1], dt.bfloat16)
            posT = ppool.tile([128, 2, NCOL1], dt.bfloat16)
            pooled = ppool.tile([128, 3 * NCOL2], dt.bfloat16)
            c2a = ppool.tile([100, 2 * NCOL2], dt.bfloat16)
            h_sb = wpool.tile([100, 3, BC], dt.float32)
            stg = wpool.tile([SS, STG_SLOTS * 256], dt.bfloat16)

            # zero pads: staging pad rows; plane lead/tail; pooled/c2a gaps
            nc.gpsimd.memset(stg[96:SS, :], 0)
            for pl in (wordT, wghT, posT):
                nc.gpsimd.memset(pl[:, :, 0:LEAD], 0)
                nc.gpsimd.memset(pl[:, :, LEAD + BC * SS :], 0)
            nc.gpsimd.memset(pooled, 0)
            nc.gpsimd.memset(c2a, 0)

            with tc.tile_pool(name="psum1", bufs=1, space="PSUM") as ps1:
                for g in range(NT1):
                    cs = LEAD + W1 * g
                    # gather this group's samples into the staging ring
                    for j in range(G1):
                        s = G1 * g + j
                        slot = s % STG_SLOTS
                        nc.gpsimd.indirect_dma_start(
                            out=stg[:100, 256 * slot : 256 * slot + 256],
                            out_offset=None,
                            in_=word_t,
                            in_offset=bass.IndirectOffsetOnAxis(
                                ap=idx_sb[:100, s : s + 1], axis=0
                            ),
                        )
                    # transpose word chunks, build weighted plane
                    for c in range(2):
                        ptr = ps1.tile([128, W1], dt.bfloat16, tag="tr", bufs=2)
                        for j in range(G1):
                            s = G1 * g + j
                            slot = s % STG_SLOTS
                            nc.tensor.transpose(
                                out=ptr[:, SS * j : SS * (j + 1)],
                                in_=stg[:, 256 * slot + 128 * c :
                                        256 * slot + 128 * (c + 1)],
                                identity=ident[:SS, :SS],
                            )
                        eng = nc.vector if c == 0 else nc.scalar
                        if c == 0:
                            nc.vector.tensor_copy(
                                out=wordT[:, c, cs : cs + W1], in_=ptr)
                        else:
                            nc.scalar.copy(out=wordT[:, c, cs : cs + W1], in_=ptr)
                        nc.vector.tensor_tensor(
                            out=wghT[:, c, cs : cs + W1],
                            in0=wordT[:, c, cs : cs + W1],
                            in1=sdpb_sb[:, cs : cs + W1],
                            op=mybir.AluOpType.mult,
                        )
                    # pos planes via one-hot matmul
                    for tb, (oh_d, pe_sb) in enumerate(
                        ((oh1, p1e_sb), (oh2, p2e_sb))
                    ):
                        pps = ps1.tile([128, W1], dt.float32, tag="pos", bufs=2)
                        for kc in range(2):
                            ohs = spool.tile([102, W1], dt.bfloat16, tag="oh")
                            nc.sync.dma_start(
                                out=ohs,
                                in_=oh_d[102 * kc : 102 * (kc + 1), cs : cs + W1],
                            )
                            nc.tensor.matmul(
                                out=pps,
                                lhsT=pe_sb[:, 128 * kc : 128 * (kc + 1)],
                                rhs=ohs,
                                start=(kc == 0),
                                stop=(kc == 1),
                            )
                        if tb == 0:
                            nc.vector.tensor_copy(
                                out=posT[:, tb, cs : cs + W1], in_=pps)
                        else:
                            nc.scalar.copy(out=posT[:, tb, cs : cs + W1], in_=pps)
                    # conv1 + relu + maxpool2
                    planes = (wordT, posT, wghT)
                    for ci, taps in enumerate(plan):
                        acc = ps1.tile([128, W1], dt.float32, tag="acc", bufs=3)
                        n = len(taps)
                        for j, (wi, plane, c, d) in enumerate(taps):
                            nc.tensor.matmul(
                                out=acc,
                                lhsT=w1_sb[:, 128 * wi : 128 * (wi + 1)],
                                rhs=planes[plane][:, c, cs + d : cs + d + W1],
                                start=(j == 0),
                                stop=(j == n - 1),
                            )
                        tmp = wpool.tile([128, G1, 56], dt.float32, tag="tmp",
                                         bufs=2)
                        nc.vector.tensor_reduce(
                            out=tmp,
                            in_=acc.rearrange("p (a c two) -> p a c two",
                                              a=G1, two=2),
                            axis=mybir.AxisListType.X,
                            op=mybir.AluOpType.max,
                        )
                        nc.scalar.activation(
                            out=pooled[:, ci * NCOL2 + 2 : ci * NCOL2 + 2 + 3328]
                            .rearrange("p (a b) -> p a b", b=SS2)
                            [:, G1 * g : G1 * (g + 1), 0:50],
                            in_=tmp[:, :, :50],
                            func=mybir.ActivationFunctionType.Relu,
                            bias=b1_sb[:, ci : ci + 1],
                        )

            with tc.tile_pool(name="psum2", bufs=1, space="PSUM") as ps2:
                # conv2a (1x1, 300 -> 200), no relu
                for t2 in range(NT2):
                    cst = 2 + W2 * t2
                    for mc in range(2):
                        acc2 = ps2.tile([128, W2], dt.float32, tag="a2", bufs=2)
                        for kc in range(3):
                            nc.tensor.matmul(
                                out=acc2,
                                lhsT=w2a_sb[:, 128 * (kc * 2 + mc) :
                                            128 * (kc * 2 + mc + 1)],
                                rhs=pooled[0:100,
                                           kc * NCOL2 + cst : kc * NCOL2 + cst + W2],
                                start=(kc == 0),
                                stop=(kc == 2),
                            )
                        nc.vector.tensor_scalar_add(
                            c2a[:, mc * NCOL2 + cst : mc * NCOL2 + cst + W2]
                            .rearrange("p (a b) -> p a b", b=SS2)[:, :, 0:50],
                            acc2[0:100, :]
                            .rearrange("p (a b) -> p a b", b=SS2)[:, :, 0:50],
                            b2a_sb[:, mc : mc + 1],
                        )
                # conv2b (1x3, 200 -> 300) + relu + maxpool50
                for t2 in range(NT2):
                    cst = 2 + W2 * t2
                    for mc in range(3):
                        acc3 = ps2.tile([128, W2], dt.float32, tag="a3", bufs=2)
                        first = True
                        for dw in range(3):
                            d = dw - 1
                            for kc in range(2):
                                nc.tensor.matmul(
                                    out=acc3,
                                    lhsT=w2b_sb[:, 128 * ((dw * 2 + kc) * 3 + mc) :
                                                128 * ((dw * 2 + kc) * 3 + mc + 1)],
                                    rhs=c2a[:, kc * NCOL2 + cst + d :
                                            kc * NCOL2 + cst + d + W2],
                                    start=first,
                                    stop=(dw == 2 and kc == 1),
                                )
                                first = False
                        red = wpool.tile([100, G2], dt.float32, tag="red", bufs=2)
                        nc.vector.tensor_reduce(
                            out=red,
                            in_=acc3[0:100, :]
                            .rearrange("p (a b) -> p a b", b=SS2)[:, :, 0:50],
                            axis=mybir.AxisListType.X,
                            op=mybir.AluOpType.max,
                        )
                        nc.scalar.activation(
                            out=h_sb[:, mc, G2 * t2 : G2 * (t2 + 1)],
                            in_=red,
                            func=mybir.ActivationFunctionType.Relu,
                            bias=b2b_sb[:, mc : mc + 1],
                        )
                # dense layers (fp32)
                pd1 = ps2.tile([100, BC], dt.float32, tag="d1")
                for kc in range(3):
                    nc.tensor.matmul(
                        out=pd1,
                        lhsT=d1w_sb[:, 100 * kc : 100 * (kc + 1)],
                        rhs=h_sb[:, kc, :],
                        start=(kc == 0),
                        stop=(kc == 2),
                    )
                s1 = wpool.tile([100, BC], dt.float32)
                nc.scalar.activation(
                    out=s1, in_=pd1,
                    func=mybir.ActivationFunctionType.Relu, bias=d1b_sb)
                pd2 = ps2.tile([50, BC], dt.float32, tag="d2")
                nc.tensor.matmul(out=pd2, lhsT=d2w_sb, rhs=s1,
                                 start=True, stop=True)
                s2 = wpool.tile([50, BC], dt.float32)
                nc.scalar.activation(
                    out=s2, in_=pd2,
                    func=mybir.ActivationFunctionType.Relu, bias=d2b_sb)
                pd3 = ps2.tile([15, BC], dt.float32, tag="d3")
                nc.tensor.matmul(out=pd3, lhsT=d3w_sb, rhs=s2,
                                 start=True, stop=True)
                o_sb = wpool.tile([15, BC], dt.float32)
                nc.vector.tensor_scalar_add(o_sb, pd3, d3b_sb)
                nc.sync.dma_start(out=out_d, in_=o_sb)

    nc.compile()
    return nc


def _pack_weights(inp):
    """Host-side static weight packing (shared across cores)."""
    f32 = np.float32
    tiles = []
    for w, k, pw in ((inp["w11"], 1, 0), (inp["w13"], 3, 1), (inp["w15"], 5, 2)):
        w = np.asarray(w, f32)
        for dw in range(k):
            for plane in range(3):
                for c in range(2):
                    t = np.zeros((128, 128), f32)
                    if plane in (0, 2):
                        lo, hi = (0, 128) if c == 0 else (128, 250)
                    else:
                        lo, hi = (0, 125) if c == 0 else (125, 250)
                    t[: hi - lo, :100] = w[:, lo:hi, plane, dw].T
                    tiles.append(t)
    w1 = np.stack(tiles).transpose(1, 0, 2).reshape(128, -1).astype(BF16)

    def pos_pack(pe):
        pe = np.asarray(pe, f32)
        p = np.zeros((204, 128), f32)
        p[:, :125] = pe
        return p.reshape(2, 102, 128).transpose(1, 0, 2).reshape(102, -1).astype(BF16)

    p1e = pos_pack(inp["pos1_emb"])
    p2e = pos_pack(inp["pos2_emb"])

    w2a_w = np.asarray(inp["w2a"], f32)[:, :, 0, 0]          # [200, 300]
    t2a = []
    for kc in range(3):
        for mc in range(2):
            t = np.zeros((100, 128), f32)
            t[:, :100] = w2a_w[100 * mc : 100 * (mc + 1),
                               100 * kc : 100 * (kc + 1)].T
            t2a.append(t)
    w2a = np.stack(t2a).transpose(1, 0, 2).reshape(100, -1).astype(BF16)

    w2b_w = np.asarray(inp["w2b"], f32)[:, :, 0, :]          # [300, 200, 3]
    t2b = []
    for dw in range(3):
        for kc in range(2):
            for mc in range(3):
                t = np.zeros((100, 128), f32)
                t[:, :100] = w2b_w[100 * mc : 100 * (mc + 1),
                                   100 * kc : 100 * (kc + 1), dw].T
                t2b.append(t)
    w2b = np.stack(t2b).transpose(1, 0, 2).reshape(100, -1).astype(BF16)

    d1w = np.asarray(inp["d1w"], f32).reshape(3, 100, 100)
    d1w = d1w.transpose(1, 0, 2).reshape(100, 300)

    b1 = np.zeros((128, 3), f32)
    for i, k in enumerate(("b11", "b13", "b15")):
        b1[:100, i] = np.asarray(inp[k], f32)
    b2a = np.asarray(inp["b2a"], f32).reshape(2, 100).T.copy()
    b2b = np.asarray(inp["b2b"], f32).reshape(3, 100).T.copy()

    return {
        "w1": w1, "p1e": p1e, "p2e": p2e, "w2a": w2a, "w2b": w2b,
        "d1w": np.ascontiguousarray(d1w),
        "d2w": np.asarray(inp["d2w"], f32),
        "d3w": np.asarray(inp["d3w"], f32),
        "b1": b1, "b2a": b2a, "b2b": b2b,
        "d1b": np.asarray(inp["d1b"], f32).reshape(100, 1),
        "d2b": np.asarray(inp["d2b"], f32).reshape(50, 1),
        "d3b": np.asarray(inp["d3b"], f32).reshape(15, 1),
    }


def _col_ids():
    s = np.arange(BC)[:, None]
    w = np.arange(L)[None, :]
    return (LEAD + SS * s + w).ravel()          # [BC*L]


def _make_in_maps(inputs):
    tok = np.asarray(inputs["tokenMatrix"]).astype(np.int64)
    pos1 = np.asarray(inputs["pos1Matrix"]).astype(np.int64)
    pos2 = np.asarray(inputs["pos2Matrix"]).astype(np.int64)
    sdp = np.asarray(inputs["sdpMatrix"], dtype=np.float32)

    word = np.zeros((V, 256), np.float32)
    word[:, :250] = np.asarray(inputs["word_emb"], np.float32)
    word = word.astype(BF16)

    shared = _pack_weights(inputs)
    cols = _col_ids()

    in_maps = []
    for c in range(NCORES):
        sl = slice(BC * c, BC * (c + 1))
        tok_c = np.zeros((SS, BC), np.int32)
        tok_c[:100, :] = tok[sl].T
        sdp_row = np.zeros(NCOL1, np.float32)
        sdp_row[cols] = sdp[sl].ravel()
        sdpb = np.broadcast_to(sdp_row.astype(BF16), (128, NCOL1)).copy()

        def onehot(pm):
            oh = np.zeros((204, NCOL1), BF16)
            oh[pm[sl].ravel(), cols] = 1
            return oh

        in_maps.append({
            "word_t": word, "tok_idx": tok_c, "sdpb": sdpb,
            "oh1": onehot(pos1), "oh2": onehot(pos2), **shared,
        })
    return in_maps


def _get_nc():
    if "nc" not in _cache:
        _cache["nc"] = _build_nc()
    return _cache["nc"]


def run_cores(inputs, trace=False):
    nc = _get_nc()
    in_maps = _make_in_maps(inputs)
    return run_bass_kernel_spmd(nc, in_maps, list(range(NCORES)), trace=trace)


def kernel(**inputs):
    res = run_cores(inputs, trace=False)
    out = np.empty((B, 15), np.float32)
    for c in range(NCORES):
        out[BC * c : BC * (c + 1)] = res.results[c]["out"].T
    return out
